# revision 1
# baseline (speedup 1.0000x reference)
"""ClusterNormZCA Trainium2 kernel.

Full inputs x[256, 64, 4096] f32 -> Z[256, 64, 4096] f32.
Sharded over batch across 8 NeuronCores (32 batches/core, zero comm).

Per core, batches are processed in pairs ("tiles" of [128, 4096] = 2x64
rows). Covariance Gram matrices are accumulated in bf16 on the PE from
on-chip transposed copies; Ledoit-Wolf shrinkage stats are computed with
a tiny PE matmul; the inverse square root uses a Newton-Schulz iteration
on the trace-normalized shrunk covariance (spectrum ~= 1, converges in
2 steps after the exact first step); whitening applies S = I + Delta
with the Delta matmul in bf16 and the identity path in fp32.
"""

import sys

for _p in ("/opt/trn_rl_repo", "/root/.axon_site/_ro/trn_rl_repo"):
    if _p not in sys.path:
        sys.path.append(_p)

import numpy as np

B, C, M = 256, 64, 4096
N_CORES = 8
B_CORE = B // N_CORES          # 32
NTILES = B_CORE // 2           # 16 pairs per core
NCHUNK = M // 128              # 32 transpose chunks per tile
NSLICE = M // 512              # 8 whitening slices per tile
C1 = float(M - 2) / float(M)   # (n-2)/n
C2 = float(M + 2)              # n+2
RINV_M = 1.0 / float(M)

_CACHE = {}


def _consts_np():
    ident = np.eye(128, dtype=np.float32)
    i15 = (1.5 * np.eye(128)).astype(np.float32)
    maskblk = np.zeros((128, 128), dtype=np.float32)
    maskblk[:64, :64] = 1.0
    maskblk[64:, 64:] = 1.0
    bcast = np.zeros((2, 128), dtype=np.float32)
    bcast[0, :64] = 1.0
    bcast[1, 64:] = 1.0
    halves = np.zeros((128, 2), dtype=np.float32)
    halves[:64, 0] = 1.0
    halves[64:, 1] = 1.0
    import ml_dtypes

    identbf = ident.astype(ml_dtypes.bfloat16)
    return {
        "identbf": identbf,
        "identf": ident,
        "i15": i15,
        "maskblk": maskblk,
        "bcast": bcast,
        "halves": halves,
    }


def _build(ntiles=NTILES):
    import concourse.bacc as bacc
    import concourse.mybir as mybir
    from concourse.tile import TileContext

    f32 = mybir.dt.float32
    bf16 = mybir.dt.bfloat16
    AF = mybir.ActivationFunctionType
    OP = mybir.AluOpType
    AX = mybir.AxisListType

    nc = bacc.Bacc("TRN2", target_bir_lowering=False, debug=False)
    X = nc.declare_dram_parameter("x", [2 * ntiles, C, M], f32, isOutput=False)
    O = nc.declare_dram_parameter("z", [2 * ntiles, C, M], f32, isOutput=True)
    CONST = {
        "identbf": nc.declare_dram_parameter("identbf", [128, 128], bf16, isOutput=False),
        "identf": nc.declare_dram_parameter("identf", [128, 128], f32, isOutput=False),
        "i15": nc.declare_dram_parameter("i15", [128, 128], f32, isOutput=False),
        "maskblk": nc.declare_dram_parameter("maskblk", [128, 128], f32, isOutput=False),
        "bcast": nc.declare_dram_parameter("bcast", [2, 128], f32, isOutput=False),
        "halves": nc.declare_dram_parameter("halves", [128, 2], f32, isOutput=False),
    }

    with TileContext(nc) as tc:
        with (
            tc.tile_pool(name="cpool", bufs=1) as cpool,
            tc.tile_pool(name="xin", bufs=4) as xin_p,
            tc.tile_pool(name="x16", bufs=3) as x16_p,
            tc.tile_pool(name="ybuf", bufs=2) as ybuf_p,
            tc.tile_pool(name="zout", bufs=2) as zout_p,
            tc.tile_pool(name="mid", bufs=2) as mid_p,
            tc.tile_pool(name="tiny", bufs=2) as tiny_p,
            tc.tile_pool(name="tvp", bufs=3) as tvp_p,
            tc.tile_pool(name="pst", bufs=1, space="PSUM") as pst_p,
            tc.tile_pool(name="gps", bufs=2, space="PSUM") as gps_p,
            tc.tile_pool(name="sml", bufs=1, space="PSUM") as sml_p,
            tc.tile_pool(name="nsp", bufs=1, space="PSUM") as nsp_p,
            tc.tile_pool(name="wps", bufs=2, space="PSUM") as wps_p,
        ):
            cb = {}
            for nm, hd in CONST.items():
                shp = list(hd.shape)
                dt = hd.dtype
                t = cpool.tile(shp, dt, name=f"c_{nm}")
                nc.sync.dma_start(out=t, in_=hd[:])
                cb[nm] = t
            identbf, identf = cb["identbf"], cb["identf"]
            i15, maskblk = cb["i15"], cb["maskblk"]
            bcast, halves = cb["bcast"], cb["halves"]

            for t in range(ntiles):
                # ---- load pair of batches ----
                xt = xin_p.tile([128, M], f32, name="xt")
                nc.sync.dma_start(
                    out=xt, in_=X[2 * t : 2 * t + 2].rearrange("b c m -> (b c) m")
                )

                # ---- cast to bf16 + fp32 row sums (ACT) ----
                x16 = x16_p.tile([128, M], bf16, name="x16")
                sacc = tiny_p.tile([128, 1], f32, name="sacc")
                nc.scalar.activation(x16, xt, AF.Copy, accum_out=sacc)

                # ---- transposes (PE, bf16) + Y copyback (ACT) ----
                ybuf = ybuf_p.tile([128, M], bf16, name="ybuf")
                for g in range(NCHUNK // 8):
                    pst = pst_p.tile([128, 1024], f32, name="pst")
                    for j in range(8):
                        k = 8 * g + j
                        nc.tensor.matmul(
                            pst[:, 128 * j : 128 * (j + 1)],
                            x16[:, 128 * k : 128 * (k + 1)],
                            identbf,
                            start=True,
                            stop=True,
                        )
                    nc.scalar.copy(ybuf[:, 1024 * g : 1024 * (g + 1)], pst)

                # ---- Gram accumulation (PE, bf16) ----
                gps = gps_p.tile([128, 128], f32, name="gps")
                for k in range(NCHUNK):
                    yk = ybuf[:, 128 * k : 128 * (k + 1)]
                    nc.tensor.matmul(gps, yk, yk, start=(k == 0), stop=False)

                # rank-1 mean correction: G -= s s^T / M (cross blocks
                # polluted, but masked out downstream)
                sml = sml_p.tile([128, 512], f32, name="sml")
                srp = sml[0:1, 0:128]
                nc.tensor.transpose(srp, sacc, identf)
                sneg = tiny_p.tile([1, 128], bf16, name="sneg")
                nc.scalar.activation(sneg, srp, AF.Identity, scale=-RINV_M)
                s16 = tiny_p.tile([1, 128], bf16, name="s16")
                nc.scalar.copy(s16, srp)
                nc.tensor.matmul(gps, sneg, s16, start=False, stop=True)

                # ---- shrinkage stats ----
                mg = mid_p.tile([128, 128], f32, name="mg")
                nc.vector.tensor_tensor(out=mg, in0=gps, in1=maskblk, op=OP.mult)
                dtmp = mid_p.tile([128, 128], f32, name="dtmp")
                nc.gpsimd.tensor_tensor(out=dtmp, in0=mg, in1=identf, op=OP.mult)
                statc = tiny_p.tile([128, 2], f32, name="statc")
                nc.vector.tensor_reduce(
                    out=statc[:, 0:1], in_=dtmp, axis=AX.X, op=OP.add
                )
                sqt = mid_p.tile([128, 128], f32, name="sqt")
                nc.gpsimd.tensor_tensor(out=sqt, in0=mg, in1=mg, op=OP.mult)
                nc.vector.tensor_reduce(
                    out=statc[:, 1:2], in_=sqt, axis=AX.X, op=OP.add
                )
                # [2,2]: row h = (D, SQ) of batch h
                stp = sml[0:2, 256:258]
                nc.tensor.matmul(stp, halves, statc, start=True, stop=True)
                st = tiny_p.tile([2, 2], f32, name="st")
                nc.vector.tensor_copy(st, stp)

                # rho chain on [2,1]
                D = st[:, 0:1]
                SQ = st[:, 1:2]
                dsq = tiny_p.tile([2, 8], f32, name="dsq")
                nc.vector.tensor_tensor(out=dsq[:, 0:1], in0=D, in1=D, op=OP.mult)
                nc.vector.scalar_tensor_tensor(
                    out=dsq[:, 1:2], in0=SQ, scalar=C1, in1=dsq[:, 0:1],
                    op0=OP.mult, op1=OP.add,
                )  # num
                nc.vector.scalar_tensor_tensor(
                    out=dsq[:, 2:3], in0=dsq[:, 0:1], scalar=-1.0 / 64.0,
                    in1=SQ, op0=OP.mult, op1=OP.add,
                )  # den0
                nc.vector.reciprocal(dsq[:, 3:4], dsq[:, 2:3])
                nc.vector.tensor_tensor(
                    out=dsq[:, 4:5], in0=dsq[:, 1:2], in1=dsq[:, 3:4], op=OP.mult
                )
                scl3 = tiny_p.tile([2, 3], f32, name="scl3")
                nc.vector.tensor_scalar(
                    out=scl3[:, 1:2], in0=dsq[:, 4:5], scalar1=1.0 / C2,
                    op0=OP.mult, scalar2=1.0, op1=OP.min,
                )  # rho
                nc.vector.tensor_scalar(
                    out=dsq[:, 5:6], in0=scl3[:, 1:2], scalar1=-64.0,
                    op0=OP.mult, scalar2=64.0, op1=OP.add,
                )  # 64(1-rho)
                nc.vector.reciprocal(dsq[:, 6:7], D)
                nc.vector.tensor_tensor(
                    out=scl3[:, 0:1], in0=dsq[:, 5:6], in1=dsq[:, 6:7], op=OP.mult
                )  # s1 = 64(1-rho)/D
                nc.scalar.sqrt(dsq[:, 7:8], dsq[:, 6:7])
                nc.scalar.mul(scl3[:, 2:3], dsq[:, 7:8], 512.0)  # rsc = 512/sqrt(D)

                # broadcast (s1, rho, rsc) to [128,3]
                bps = sml[:, 384:387]
                nc.tensor.matmul(bps, bcast, scl3, start=True, stop=True)
                bcols = tiny_p.tile([128, 3], f32, name="bcols")
                nc.vector.tensor_copy(bcols, bps)
                s1v = bcols[:, 0:1]
                rhov = bcols[:, 1:2]
                rscv = bcols[:, 2:3]

                # ---- Ahat = s1*mg + rho*I ; X1 = 1.5I - 0.5*Ahat ----
                irho = mid_p.tile([128, 128], f32, name="irho")
                nc.scalar.activation(irho, identf, AF.Identity, scale=rhov)
                ahat = mid_p.tile([128, 128], f32, name="ahat")
                nc.vector.scalar_tensor_tensor(
                    out=ahat, in0=mg, scalar=s1v, in1=irho, op0=OP.mult, op1=OP.add
                )
                xcur = mid_p.tile([128, 128], f32, name="xcur")
                nc.vector.scalar_tensor_tensor(
                    out=xcur, in0=ahat, scalar=-0.5, in1=i15, op0=OP.mult, op1=OP.add
                )

                # ---- Newton-Schulz iterations ----
                for it in range(2):
                    p1 = nsp_p.tile([128, 128], f32, name="nspt", tag="nspt")
                    nc.tensor.matmul(p1, xcur, xcur, start=True, stop=True)
                    x2 = mid_p.tile([128, 128], f32, name="x2")
                    nc.scalar.copy(x2, p1)
                    p2 = nsp_p.tile([128, 128], f32, name="nspt", tag="nspt")
                    nc.tensor.matmul(p2, ahat, x2, start=True, stop=True)
                    u = mid_p.tile([128, 128], f32, name="u")
                    nc.vector.scalar_tensor_tensor(
                        out=u, in0=p2, scalar=-0.5, in1=i15, op0=OP.mult, op1=OP.add
                    )
                    p3 = nsp_p.tile([128, 128], f32, name="nspt", tag="nspt")
                    nc.tensor.matmul(p3, xcur, u, start=True, stop=True)
                    xcur = mid_p.tile([128, 128], f32, name="xcur")
                    if it == 0:
                        nc.scalar.copy(xcur, p3)
                    else:
                        nc.scalar.activation(xcur, p3, AF.Identity, scale=rscv)

                S = xcur  # = Ahat^{-1/2} * rsc (block-diagonal pair)
                delta = mid_p.tile([128, 128], bf16, name="delta")
                nc.gpsimd.tensor_tensor(out=delta, in0=S, in1=identf, op=OP.subtract)

                # v = S @ mu ; negv = -v
                mu = tiny_p.tile([128, 1], f32, name="mu")
                nc.scalar.mul(mu, sacc, RINV_M)
                vpt = wps_p.tile([128, 512], f32, name="wps", tag="wps")
                vps = vpt[:, 0:1]
                nc.tensor.matmul(vps, S, mu, start=True, stop=True)
                negv = tiny_p.tile([128, 1], f32, name="negv")
                nc.scalar.activation(negv, vps, AF.Identity, scale=-1.0)

                # ---- whitening + fused output ----
                zt = zout_p.tile([128, M], f32, name="zt")
                for s in range(NSLICE):
                    sl = slice(512 * s, 512 * (s + 1))
                    wps = wps_p.tile([128, 512], f32, name="wps", tag="wps")
                    nc.tensor.matmul(wps, delta, x16[:, sl], start=True, stop=True)
                    if s % 3 == 2:
                        tv = tvp_p.tile([128, 512], f32, name="tv")
                        nc.scalar.activation(
                            tv, wps, AF.Identity, bias=negv[:, 0:1], scale=1.0
                        )
                        nc.gpsimd.tensor_tensor(
                            out=zt[:, sl], in0=tv, in1=xt[:, sl], op=OP.add
                        )
                    else:
                        nc.vector.scalar_tensor_tensor(
                            out=zt[:, sl], in0=wps, scalar=negv[:, 0:1],
                            in1=xt[:, sl], op0=OP.add, op1=OP.add,
                        )
                nc.sync.dma_start(
                    out=O[2 * t : 2 * t + 2].rearrange("b c m -> (b c) m"), in_=zt
                )

    nc.compile()
    return nc


def _get_nc(ntiles=NTILES):
    key = ("nc", ntiles)
    if key not in _CACHE:
        _CACHE[key] = _build(ntiles)
    return _CACHE[key]


def _install_ntff_hook():
    """Provide antenv.axon_hooks (absent in this image) so
    run_bass_kernel_spmd(trace=True) can capture NTFF profiles."""
    import types

    import antenv

    if "antenv.axon_hooks" in sys.modules:
        return
    mod = types.ModuleType("antenv.axon_hooks")
    state = [None]
    mod.set_axon_ntff_profile_hook = lambda h: state.__setitem__(0, h)
    mod.get_axon_ntff_profile_hook = lambda: state[0]
    sys.modules["antenv.axon_hooks"] = mod
    antenv.axon_hooks = mod
    try:
        from trn_agent_boot.trn_boot import _ntff_profile_via_ctypes

        mod.set_axon_ntff_profile_hook(
            _ntff_profile_via_ctypes("/opt/axon/libaxon_pjrt.so")
        )
    except Exception:
        pass



def _patch_ldw_opt():
    import concourse.bass_utils as bu

    if getattr(bu, "_ldw_patched", False):
        return
    orig = bu.run_command

    def patched(argv, **kw):
        argv = [
            a
            if isinstance(a, str)
            else a
            for a in argv
        ]
        return orig(argv, **kw)

    bu.run_command = patched
    bu._ldw_patched = True


def _run(x, trace=False):
    from concourse.bass_utils import run_bass_kernel_spmd

    _patch_ldw_opt()
    if trace:
        _install_ntff_hook()

    nc = _get_nc()
    consts = _consts_np()
    x = np.ascontiguousarray(x, dtype=np.float32)
    in_maps = [
        {"x": x[i * B_CORE : (i + 1) * B_CORE], **consts} for i in range(N_CORES)
    ]
    res = run_bass_kernel_spmd(
        nc, in_maps, list(range(N_CORES)), trace=trace
    )
    out = np.concatenate([res.results[i]["z"] for i in range(N_CORES)], axis=0)
    return out, res


def kernel(x):
    out, _ = _run(x)
    return out

